# revision 20
# baseline (speedup 1.0000x reference)
"""Trainium2 Bass kernel for nn_MetaTwistorLNN (complex Liquid NN recurrence).

Strategy
--------
Data-parallel over batch: 8 cores x 128 batch rows each; the T=512 recurrence
runs locally per core. State kept TRANSPOSED: z tile [128(part)=h within
chunk, 512(free)] with columns [r_chk0 | i_chk0 | r_chk1 | i_chk1] so matmul
rhs operands need no transposes (contraction dim = partition dim = h).

Numerics (the recurrence is chaotic: per-step noise amplifies ~200-1000x, so
everything must be fp32-grade; empirically measured on HW):
  - all recurrence matmuls fp32 (fp32r is 13-bit mantissa -> unusable)
  - tanh(z) = 2*recip(1+exp(-2z)) - 1, with the x2 folded into host-prepped
    weights (Wz' = 2*Wz) and the -1 folded into a per-partition constant
    (-rowsum(Wz)) added in the same fused scalar_tensor_tensor op.
    exp on ACT (~1e-7..1e-5 rel), reciprocal on DVE (exact, 6e-8).
  - z_mod = sqrt(zr^2+zi^2) = exp(0.5*ln(m2)): ln+exp live in the SAME ACT
    table set (natural_log_exp_and_others) as all other ACT funcs used here,
    so no 2.7us table reloads inside the loop. sqrt's own table set has no
    exp/tanh, which is why tanh/sigmoid are restructured onto exp.
  - 1/tau = 1/(sigmoid(s)+1e-6) ~= 1+exp(-s) (exact up to 1e-6*(1+e): rel
    ~2e-6/step -> ~6e-4 final, within the fp32 impl-to-impl envelope ~2e-4).
  - DT*clip(v,+-10) = clip(DT*v,+-1); DT folded into the update STT.
  - output projection y = z_r @ W_out.T in fp16 (no feedback into the
    recurrence; rel err ~5e-4), batched 4 steps per matmul (N=512).
  - x is pre-transposed on the host ([T,IN,BC] contiguous slabs), y is
    written transposed ([T,OUT,BC]) and fixed up on the host.
"""
import sys
sys.path.insert(0, '/opt/trn_rl_repo')

import numpy as np
from contextlib import ExitStack

import concourse.bass as bass
import concourse.bacc as bacc
import concourse.mybir as mybir
from concourse import tile
from concourse.bass_utils import run_bass_kernel_spmd

f32 = mybir.dt.float32
f16 = mybir.dt.float16
AF = mybir.ActivationFunctionType
OP = mybir.AluOpType

T, B, IN, H, OUT = 512, 1024, 64, 256, 32
NCORES = 8
BC = B // NCORES            # 128 batch rows per core
P = 128                     # SBUF partitions
NCH = H // P                # 2 h-chunks
W = 2 * H                   # 512: z free width  [r0|i0|r1|i1]
U = 8                       # steps per For_i trip (even, multiple of YB)
YB = 4                      # y-projection batch (steps per y matmul)
DT_ = 0.1

_cache = {}
_DEBUG = False


def _build(T_steps, u, trace_enabled=False):
    """Build the SPMD bass program (one program, run on 8 cores)."""
    nc = bacc.Bacc("TRN2", target_bir_lowering=False)
    dbg_tensors = {}

    def dbg(name, ap, shape):
        if not _DEBUG or name in dbg_tensors:
            return
        d = nc.dram_tensor(f"dbg_{name}", list(shape), ap.dtype,
                           kind="ExternalOutput")
        dbg_tensors[name] = d
        nc.sync.dma_start(out=d[:], in_=ap)

    xT_d = nc.dram_tensor("xT", [T_steps * IN, BC], f32, kind="ExternalInput")
    wzT_d = nc.dram_tensor("wzT", [H, H], f32, kind="ExternalInput")     # (2Wz).T
    wtauT_d = nc.dram_tensor("wtauT", [H, H], f32, kind="ExternalInput")  # Wtau.T
    wxT_d = nc.dram_tensor("wxT", [IN, H], f32, kind="ExternalInput")    # Wx.T
    woutT_d = nc.dram_tensor("woutT", [H, OUT], f16, kind="ExternalInput")
    cbar_d = nc.dram_tensor("cbar", [H, 1], f32, kind="ExternalInput")   # -rowsum(Wz)+b_z (+b_x==0)
    yT_d = nc.dram_tensor("yT", [T_steps * OUT, BC], f32, kind="ExternalOutput")

    trips = T_steps // u

    with tile.TileContext(nc) as tc, ExitStack() as ctx:
        const = ctx.enter_context(tc.tile_pool(name="const", bufs=1))
        state = ctx.enter_context(tc.tile_pool(name="state", bufs=1))
        xp = ctx.enter_context(tc.tile_pool(name="xp", bufs=4))
        wk = ctx.enter_context(tc.tile_pool(name="wk", bufs=2))
        wk2 = ctx.enter_context(tc.tile_pool(name="wk2", bufs=2))
        ps_dz = ctx.enter_context(tc.tile_pool(name="ps_dz", bufs=2, space="PSUM"))
        ps_s = ctx.enter_context(tc.tile_pool(name="ps_s", bufs=2, space="PSUM"))
        ps_y = ctx.enter_context(tc.tile_pool(name="ps_y", bufs=2, space="PSUM"))

        # ---- constants (loaded once) ----
        wz = [const.tile([P, H], f32, tag=f"wz{k}", name=f"wz{k}") for k in range(NCH)]
        wtau = [const.tile([P, H], f32, tag=f"wtau{k}", name=f"wtau{k}") for k in range(NCH)]
        wx = const.tile([IN, H], f32, tag="wx")
        wout = [const.tile([P, OUT], f16, tag=f"wout{k}", name=f"wout{k}") for k in range(NCH)]
        cbar = [const.tile([P, 1], f32, tag=f"cbar{m}", name=f"cbar{m}") for m in range(NCH)]
        lnbias = const.tile([P, 1], f32, tag="lnbias")
        zb = const.tile([P, 1], f32, tag="zb")
        for k in range(NCH):
            nc.sync.dma_start(out=wz[k][:], in_=wzT_d[k * P:(k + 1) * P, :])
            nc.sync.dma_start(out=wtau[k][:], in_=wtauT_d[k * P:(k + 1) * P, :])
            nc.sync.dma_start(out=wout[k][:], in_=woutT_d[k * P:(k + 1) * P, :])
            nc.sync.dma_start(out=cbar[k][:], in_=cbar_d[k * P:(k + 1) * P, :])
        nc.sync.dma_start(out=wx[:], in_=wxT_d[:])
        nc.vector.memset(lnbias[:], 1e-38)   # ln(m2+1e-38): ln(0) guard
        nc.vector.memset(zb[:], 0.0)

        # ---- state ----
        zA = state.tile([P, W], f32, tag="zA")
        zB = state.tile([P, W], f32, tag="zB")
        ystage = [state.tile([P, YB * P], f16, tag=f"ystage{k}", name=f"ystage{k}") for k in range(NCH)]
        nc.vector.memset(zA[:], 0.0)

        def step(trip_sym, j):
            """One recurrence step. trip_sym: symbolic trip index; j: unrolled pos."""
            t_sym = trip_sym * u + j
            z = zA if j % 2 == 0 else zB
            znew = zB if j % 2 == 0 else zA

            # x_t load (prefetched via pool bufs)
            xt = xp.tile([IN, BC], f32, tag="xt")
            nc.sync.dma_start(out=xt[:], in_=xT_d[bass.ts(t_sym, IN), :])

            # |z|^2: sq = z*z (ACT square), m2 = sq_r + sq_i (GPSIMD, strided)
            sq = wk.tile([P, W], f32, tag="sq")
            nc.scalar.activation(sq[:], z[:], AF.Square, bias=zb[:])
            m2 = wk2.tile([P, H], f32, tag="m2")
            sq4 = sq[:].rearrange("p (c two b) -> p c two b", c=NCH, two=2, b=P)
            m2v = m2[:].rearrange("p (c b) -> p c b", c=NCH)
            nc.gpsimd.tensor_tensor(m2v, sq4[:, :, 0, :], sq4[:, :, 1, :], OP.add)

            # tanh path: E = exp(-2z); th = 2*recip(E+1) - 1 (explicit, centered:
            # folding the -1 into the weights causes catastrophic cancellation)
            E = wk.tile([P, W], f32, tag="E")
            nc.scalar.activation(E[:], z[:], AF.Exp, bias=zb[:], scale=-2.0)
            den = wk.tile([P, W], f32, tag="den")
            nc.vector.tensor_scalar(den[:], E[:], 1.0, None, OP.add)
            r = wk.tile([P, W], f32, tag="r")
            nc.vector.reciprocal(r[:], den[:])
            th = wk.tile([P, W], f32, tag="th")
            nc.gpsimd.tensor_scalar(th[:], r[:], 2.0, -1.0, OP.mult, OP.add)

            # z_mod = exp(0.5*ln(m2+eps))
            L = wk2.tile([P, H], f32, tag="L")
            nc.scalar.activation(L[:], m2[:], AF.Ln, bias=lnbias[:])
            zmod = wk2.tile([P, H], f32, tag="zmod")
            nc.scalar.activation(zmod[:], L[:], AF.Exp, bias=zb[:], scale=0.5)

            # tau matmuls: s = Wtau @ z_mod   [m-chunk 128p, 128b]
            psum_s = ps_s.tile([P, H], f32, tag="ps_s")
            for m in range(NCH):
                for k in range(NCH):
                    nc.tensor.matmul(
                        psum_s[:, m * P:(m + 1) * P],
                        wtau[k][:, m * P:(m + 1) * P],
                        zmod[:, k * P:(k + 1) * P],
                        start=(k == 0), stop=(k == NCH - 1))
            # e = exp(-s); 1/tau ~= 1+e
            e = wk2.tile([P, H], f32, tag="e")
            nc.scalar.activation(e[:], psum_s[:], AF.Exp, bias=zb[:], scale=-1.0)

            # dz matmuls: psum = Wz @ th (+ Ux into r-halves)
            psum = ps_dz.tile([P, W], f32, tag="ps_dz")
            for m in range(NCH):
                sl = slice(m * 2 * P, (m + 1) * 2 * P)
                nc.tensor.matmul(psum[:, sl], wz[0][:, m * P:(m + 1) * P],
                                 th[:, 0:2 * P], start=True, stop=False)
                nc.tensor.matmul(psum[:, m * 2 * P:m * 2 * P + P],
                                 wx[:, m * P:(m + 1) * P], xt[:],
                                 start=False, stop=False)
                nc.tensor.matmul(psum[:, sl], wz[1][:, m * P:(m + 1) * P],
                                 th[:, 2 * P:4 * P], start=False, stop=True)

            # t = (psum + cbar) - z ; w = (e+1) * t ; c = clip(w,+-1)
            tt = wk.tile([P, W], f32, tag="tt")
            ww = wk.tile([P, W], f32, tag="ww")
            for m in range(NCH):
                sl = slice(m * 2 * P, (m + 1) * 2 * P)
                nc.vector.scalar_tensor_tensor(
                    tt[:, sl], psum[:, sl], cbar[m][:, 0:1], z[:, sl],
                    OP.add, OP.subtract)
                ev = e[:, m * P:(m + 1) * P].unsqueeze(1).broadcast_to((P, 2, P))
                tv = tt[:, sl].rearrange("p (two b) -> p two b", two=2)
                wv = ww[:, sl].rearrange("p (two b) -> p two b", two=2)
                nc.vector.scalar_tensor_tensor(wv, ev, 1.0, tv, OP.add, OP.mult)
            cc = wk.tile([P, W], f32, tag="cc")
            nc.gpsimd.tensor_scalar(cc[:], ww[:], 10.0, -10.0, OP.min, OP.max)
            # znew = z + 0.1*c
            nc.vector.scalar_tensor_tensor(znew[:], cc[:], DT_, z[:],
                                           OP.mult, OP.add)
            if j == 0:
                dbg("sq", sq[:], (P, W)); dbg("m2", m2[:], (P, H))
                dbg("E", E[:], (P, W)); dbg("r", r[:], (P, W))
                dbg("zmod", zmod[:], (P, H)); dbg("e", e[:], (P, H))
                dbg("tt", tt[:], (P, W)); dbg("ww", ww[:], (P, W))
                dbg("cc", cc[:], (P, W)); dbg("znew", znew[:], (P, W))

            # stage z_r (post-update) for the batched fp16 y projection
            yslot = j % YB
            for k in range(NCH):
                nc.vector.tensor_copy(
                    ystage[k][:, yslot * P:(yslot + 1) * P],
                    znew[:, k * 2 * P:k * 2 * P + P])

            if yslot == YB - 1:
                # group index g: rows [g*YB*OUT, (g+1)*YB*OUT) of yT
                gsym = trip_sym * (u // YB) + (j // YB)
                psy = ps_y.tile([OUT, YB * P], f32, tag="ps_y")
                for k in range(NCH):
                    nc.tensor.matmul(psy[:], wout[k][:], ystage[k][:],
                                     start=(k == 0), stop=(k == NCH - 1))
                ysb = wk2.tile([OUT, YB * P], f32, tag="ysb")
                nc.scalar.copy(ysb[:], psy[:])
                # store transposed: yT[(g*YB+jj)*OUT + o, b] = ysb[o, jj*P+b]
                dst = yT_d[bass.ts(gsym, YB * OUT), :] \
                    .rearrange("(jj o) b -> o jj b", jj=YB, o=OUT)
                src = ysb[:].rearrange("o (jj b) -> o jj b", jj=YB)
                nc.sync.dma_start(out=dst, in_=src)

        if trips > 1:
            with tc.For_i(0, trips) as trip:
                for j in range(u):
                    step(trip, j)
        else:
            for j in range(u):
                step(0, j)

    nc.compile()
    return nc


def _prep_host(x, W_z, W_x, W_out, W_tau, b_z, b_x, b_out):
    x = np.ascontiguousarray(np.asarray(x, dtype=np.float32))
    W_z = np.asarray(W_z, dtype=np.float32)
    W_x = np.asarray(W_x, dtype=np.float32)
    W_out = np.asarray(W_out, dtype=np.float32)
    W_tau = np.asarray(W_tau, dtype=np.float32)
    b_z = np.asarray(b_z, dtype=np.float32)
    b_x = np.asarray(b_x, dtype=np.float32)

    assert not np.any(b_x), "nonzero b_x needs the split-halves cbar path"
    wzT = np.ascontiguousarray(W_z.T)
    wtauT = np.ascontiguousarray(W_tau.T)
    wxT = np.ascontiguousarray(W_x.T)
    woutT = np.ascontiguousarray(W_out.T).astype(np.float16)
    cbar = np.broadcast_to(b_z.reshape(-1), (H,)).astype(np.float32).reshape(H, 1)
    shared = {"wzT": wzT, "wtauT": wtauT, "wxT": wxT, "woutT": woutT,
              "cbar": np.ascontiguousarray(cbar)}
    in_maps = []
    for c in range(NCORES):
        xc = x[:, c * BC:(c + 1) * BC, :]                  # [T, BC, IN]
        xT = np.ascontiguousarray(xc.transpose(0, 2, 1))   # [T, IN, BC]
        m = dict(shared)
        m["xT"] = xT.reshape(T * IN, BC)
        in_maps.append(m)
    return in_maps


def _install_ntff_hook():
    """Inject antenv.axon_hooks (missing in this image) so trace=True works."""
    import types, importlib
    try:
        from antenv.axon_hooks import get_axon_ntff_profile_hook  # noqa
        return
    except ImportError:
        pass
    import antenv
    mod = types.ModuleType("antenv.axon_hooks")
    _state = {"hook": None}
    mod.set_axon_ntff_profile_hook = lambda h: _state.__setitem__("hook", h)
    mod.get_axon_ntff_profile_hook = lambda: _state["hook"]
    sys.modules["antenv.axon_hooks"] = mod
    antenv.axon_hooks = mod
    sys.path.insert(0, "/root/.axon_site/trn_agent_boot")
    try:
        import trn_boot
        hook = trn_boot._ntff_profile_via_ctypes("/opt/axon/libaxon_pjrt.so")
        mod.set_axon_ntff_profile_hook(hook)
    except Exception as ex:  # degrade to no tracing
        print(f"ntff hook install failed: {ex}")


def kernel(x, W_z, W_x, W_out, W_tau, b_z, b_x, b_out, _trace=False):
    if _trace:
        _install_ntff_hook()
    in_maps = _prep_host(x, W_z, W_x, W_out, W_tau, b_z, b_x, b_out)
    key = (T, U, _trace)
    if key not in _cache:
        _cache[key] = _build(T, U, trace_enabled=_trace)
    nc = _cache[key]
    res = run_bass_kernel_spmd(nc, in_maps, core_ids=list(range(NCORES)),
                               trace=_trace)
    kernel.last_exec_time_ns = res.exec_time_ns
    out = np.empty((T, B, OUT), dtype=np.float32)
    b_out = np.asarray(b_out, dtype=np.float32)
    for c in range(NCORES):
        yT = res.results[c]["yT"].reshape(T, OUT, BC)
        out[:, c * BC:(c + 1) * BC, :] = yT.transpose(0, 2, 1)
    if np.any(b_out):
        out += b_out
    return out
